# revision 6
# baseline (speedup 1.0000x reference)
"""Trainium2 Bass kernel for AttnApply (sliding-window weighted sum).

out[b, t, c] = sum_i padded[b, t+i, c] * weights[b, t, i]   (T=11, D=5 zero pad)

Strategy
--------
Pure data parallel over batch: 8 cores x 4 batches each.

Per core, the windowed sum is expressed as a banded matrix multiply on the
TensorEngine: for each time block of M=118 output rows,

    out[t0+m, c] = sum_k band[k, m] * in_pad[t0+k, c],   k in [0, 128)

with band[k, m] = w[t0+m, k-m] for 0 <= k-m < T (zero elsewhere).  K = M+T-1
= 128 exactly fills the PE contraction dim, so each block is ONE fp32 matmul
[128,118]^T @ [128,256] -> PSUM [118, 256], then a ScalarE copy to SBUF and a
DMA store.  The band matrices are built host-side (cheap scatter of the small
weights tensor) and streamed from DRAM; inputs are host zero-padded so edge
blocks need no special casing.
"""

import numpy as np

import concourse.bass as bass  # noqa: F401  (engine handles hang off nc)
import concourse.mybir as mybir
import concourse.tile as tile
from concourse import bacc
from concourse.bass_utils import run_bass_kernel_spmd

B, L, C, T = 32, 4096, 256, 11
D = T // 2
N_CORES = 8
B_LOC = B // N_CORES            # 4 batches per core
M = 118                         # output rows per matmul block
K = M + T - 1                   # 128 = contraction rows per block
NBLK = -(-L // M)               # 35 blocks per batch
LPAD = M * (NBLK - 1) + K       # 4140 padded input rows

_CACHE: dict = {}
LAST_RESULT = None  # BassKernelResults of the most recent run (for test.py)


def _build_nc(repeat: int = 1, bench: bool = False):
    """Build the bass program. `repeat` re-runs the whole body N times and
    `bench=True` uses internal zero-filled DRAM inputs/outputs with only a
    tiny external "tick" output — both used only for benchmarking; the
    grading path uses repeat=1, bench=False."""
    nc = bacc.Bacc(
        "TRN2",
        target_bir_lowering=False,
        debug=False,
        num_devices=N_CORES,
    )
    if bench:
        inp = nc.dram_tensor("in_int", [B_LOC, LPAD, C], mybir.dt.float32).ap()
        band = nc.dram_tensor("band_int", [B_LOC, NBLK, K, M], mybir.dt.float32).ap()
        out = nc.dram_tensor("out_int", [B_LOC, L, C], mybir.dt.float32).ap()
        tick = nc.dram_tensor(
            "tick", [1, C], mybir.dt.float32, kind="ExternalOutput"
        ).ap()
    else:
        inp = nc.dram_tensor(
            "in_pad", [B_LOC, LPAD, C], mybir.dt.float32, kind="ExternalInput"
        ).ap()
        band = nc.dram_tensor(
            "band", [B_LOC, NBLK, K, M], mybir.dt.float32, kind="ExternalInput"
        ).ap()
        out = nc.dram_tensor(
            "out", [B_LOC, L, C], mybir.dt.float32, kind="ExternalOutput"
        ).ap()
        tick = None

    with tile.TileContext(nc) as tc:
        with (
            tc.tile_pool(name="inp", bufs=6) as in_pool,
            tc.tile_pool(name="bnd", bufs=6) as bd_pool,
            tc.tile_pool(name="outp", bufs=6) as o_pool,
            tc.tile_pool(name="ps", bufs=8, space="PSUM") as ps_pool,
        ):
            if bench:
                # zero-fill internal inputs once per run (outside the repeat
                # loop; cancelled by the delta-timing method anyway)
                with tc.tile_pool(name="z", bufs=1) as z_pool:
                    z = z_pool.tile([128, 256], mybir.dt.float32, tag="z")
                    nc.gpsimd.memset(z[:, :], 0.0)
                    for b in range(B_LOC):
                        for r0 in range(0, LPAD, 128):
                            cnt = min(128, LPAD - r0)
                            nc.sync.dma_start(
                                out=inp[b, r0 : r0 + cnt, :], in_=z[:cnt, :]
                            )
                        for j in range(NBLK):
                            nc.sync.dma_start(out=band[b, j], in_=z[:, :M])
            for _rep in range(repeat):
              for b in range(B_LOC):
                for j in range(NBLK):
                    t0 = j * M
                    mv = min(M, L - t0)
                    in_t = in_pool.tile([K, C], mybir.dt.float32, tag="in")
                    nc.sync.dma_start(out=in_t[:, :], in_=inp[b, t0 : t0 + K, :])
                    bd_t = bd_pool.tile([K, M], mybir.dt.float32, tag="bd")
                    nc.sync.dma_start(out=bd_t[:, :], in_=band[b, j])
                    ps = ps_pool.tile([M, C], mybir.dt.float32, tag="ps")
                    nc.tensor.matmul(
                        ps[:, :], bd_t[:, :], in_t[:, :], start=True, stop=True
                    )
                    o_t = o_pool.tile([M, C], mybir.dt.float32, tag="o")
                    nc.scalar.copy(out=o_t[:mv, :], in_=ps[:mv, :])
                    nc.sync.dma_start(out=out[b, t0 : t0 + mv, :], in_=o_t[:mv, :])
            if tick is not None:
                nc.sync.dma_start(out=tick[:, :], in_=o_t[:1, :])
    nc.compile()
    return nc


def _prep_core(x: np.ndarray, w: np.ndarray):
    """x: [B_LOC, L, C] f32, w: [B_LOC, L, T] f32 -> (in_pad, band)."""
    in_pad = np.zeros((B_LOC, LPAD, C), np.float32)
    in_pad[:, D : D + L, :] = x
    band = np.zeros((B_LOC, NBLK, K, M), np.float32)
    jj, mm = np.meshgrid(np.arange(NBLK), np.arange(M), indexing="ij")
    tt = jj * M + mm
    v = tt < L
    jv, mv_, tv = jj[v], mm[v], tt[v]
    for tau in range(T):
        band[:, jv, mv_ + tau, mv_] = w[:, tv, tau]
    return in_pad, band


def kernel(inputs: np.ndarray, weights: np.ndarray) -> np.ndarray:
    global LAST_RESULT
    inputs = np.ascontiguousarray(np.asarray(inputs, dtype=np.float32))
    weights = np.ascontiguousarray(np.asarray(weights, dtype=np.float32))
    assert inputs.shape == (B, L, C) and weights.shape == (B, L, T)

    if "nc" not in _CACHE:
        _CACHE["nc"] = _build_nc()
    nc = _CACHE["nc"]

    in_maps = []
    for c in range(N_CORES):
        sl = slice(c * B_LOC, (c + 1) * B_LOC)
        ip, bd = _prep_core(inputs[sl], weights[sl])
        in_maps.append({"in_pad": ip, "band": bd})

    res = run_bass_kernel_spmd(nc, in_maps, core_ids=list(range(N_CORES)))
    LAST_RESULT = res
    return np.concatenate([r["out"] for r in res.results], axis=0)
